# revision 1
# baseline (speedup 1.0000x reference)
"""Trainium2 Bass kernel for nn_MatchLoss.

Reference computation:
    an, bn, cn = l1_normalize(a|b|c, dim=C)        # per (b, h, w) column
    sim_ab = einsum('bchw,bcij->bhwij', an, bn)
    sim_ac = einsum('bchw,bcij->bhwij', an, cn)
    out = mean(|sim_ac - sim_ab|)                   # scalar

Restructure (per batch, hw -> 4096):
    sim_ac - sim_ab = diag(1/na) @ (a^T @ D),  D = c*diag(1/nc) - b*diag(1/nb)
    loss_part = sum_q (1/na[q]) * sum_p |(a^T D)[q, p]|

Sharding: 8 cores = 2 batches x 4 slices of the p axis.  Each core gets
full `a` for its batch plus a 1024-column slice of b and c (packed as one
dram tensor [b0|c0|b1|c1] in 512-col chunks), computes 1/na-scaled
rowsums of |a^T D| into a (128 x 32) partial; host sums the partials.

Schedule highlights (tuned against the TRN2 TimelineSim cost model and
validated against the BIR verifier's hardware rules):
  * inputs are pre-cast to bf16 host-side and loaded via the sync (SP)
    queue only -- a DMA holds its issuing engine's sequencer for the
    whole transfer, and SP is the one engine with nothing else to do
  * PE warmup matmuls at t=0 beat the p-state ramp (0.65 -> 2.4 GHz)
  * norms are computed partition-major ([128,16] reciprocal, ~0.3us,
    vs 2.1us for a [1,2048] single-lane reciprocal), PE-transposed to
    rows (identity rhs), and broadcast across partitions with K=8
    one-hot selector matmuls
  * the dominant abs-rowsum of M = a^T D runs on the only two engines
    that can read PSUM (GPSIMD cannot): DVE tensor_reduce and ACT
    activation(Abs)+accum, 16 tiles each, strictly alternating through
    a 4-buffer PSUM ring so neither head-of-line blocks the PE
  * ACT's pre-loop idle covers |b|,|c| (abs_max is not a valid
    TensorScalar ALU op on hardware, so activation(Abs) is the only
    abs); |a| also fills ACT's pre-loop idle window
  * na matmuls + 1/na scaling ride the tail while the last reduces
    drain; the host only sums the 8 cores' (128 x 32) partials
"""

import numpy as np

try:
    import concourse.bacc as bacc
    import concourse.tile as tile
    import concourse.mybir as mybir
    from concourse import bass_utils
except ImportError:  # pragma: no cover - fallback for bare containers
    import sys

    sys.path.insert(0, "/opt/trn_rl_repo")
    import concourse.bacc as bacc
    import concourse.tile as tile
    import concourse.mybir as mybir
    from concourse import bass_utils

B, C, H, W = 2, 128, 64, 64
HW = H * W              # 4096 (q axis, and full p axis)
N_CORES = 8
PSL = HW // 4           # 1024: per-core p-slice
QT = 128                # q tile (partition dim of PSUM result)
NQT = HW // QT          # 32 q tiles
CH = 512                # matmul moving chunk (one PSUM bank of fp32)
NCHK = PSL // CH        # 2 chunks per core
SPLIT = 4               # q-tiles whose reduces run per-chunk (early start)

_F32 = mybir.dt.float32
_BF16 = mybir.dt.bfloat16
_AX = mybir.AxisListType
_AF = mybir.ActivationFunctionType
_OP = mybir.AluOpType


def _assign_engines():
    """Tile->engine assignment for the abs-rowsum reduces."""
    # GPSIMD cannot touch PSUM on real TRN2 (BIR verifier), so the M
    # reduces go to DVE ("D") and ACT ("A") only, strictly alternating to
    # avoid head-of-line blocking in the 4-buf PSUM pipeline.
    return ["A" if i % 2 == 0 else "D" for i in range(NQT)]


def _emit(tc, a_d, bc_d, oh_d, id_d, o_d):
    nc = tc.nc
    import contextlib

    fulls = _assign_engines()

    with contextlib.ExitStack() as ctx:
        ctx.enter_context(
            nc.allow_low_precision(
                reason="bf16 matmul inputs; accumulation stays fp32"
            )
        )
        sb = ctx.enter_context(tc.tile_pool(name="sb", bufs=1))

        A = sb.tile([C, HW], _BF16)
        bc = sb.tile([C, 2 * PSL], _BF16)      # [b0|c0|b1|c1] 512-col chunks
        absBC = sb.tile([C, 2 * PSL], _BF16)
        absA = sb.tile([C, HW], _BF16)
        D = sb.tile([C, PSL], _BF16)
        t1a = sb.tile([C, CH], _BF16)
        t2a = sb.tile([C, CH], _BF16)
        t1b = sb.tile([C, CH], _BF16)
        t2b = sb.tile([C, CH], _BF16)
        ones_col = sb.tile([C, 1], _BF16)
        zeros_w = sb.tile([C, 256], _BF16)
        rr = sb.tile([C, 16], _BF16)           # 1/norm, partition-major
        rrT0 = sb.tile([8, C], _BF16)          # chunk-0 norms as rows
        rrT1 = sb.tile([8, C], _BF16)          # chunk-1 norms as rows
        onehots = sb.tile([8, 8 * QT], _BF16)  # K=8 row selectors (DMA'd const)
        ident = sb.tile([C, C], _BF16)         # identity for PE transpose
        rna = sb.tile([C, NQT], _F32)
        rs_d = sb.tile([C, NQT], _F32)
        rs_a = sb.tile([C, NQT], _F32)
        sum1 = sb.tile([C, NQT], _F32)
        res = sb.tile([C, NQT], _F32)
        trash_a = sb.tile([C, PSL], _BF16)

        # --- t=0: DMA issue + memsets + PE warmup -------------------------
        # all loads on the sync (SP) queue: a DMA holds its issuing
        # engine's sequencer for the whole transfer, and SP is the only
        # engine with nothing else to do
        nc.sync.dma_start(bc[:, 0:1024], bc_d[:, 0:1024])
        nc.sync.dma_start(bc[:, 1024:2048], bc_d[:, 1024:2048])
        nc.sync.dma_start(ident[:], id_d[:, :])
        nc.sync.dma_start(onehots[:], oh_d[:, 0 : 8 * QT])
        nc.sync.dma_start(A[:, 0:2048], a_d[:, 0:2048])
        nc.sync.dma_start(A[:, 2048:4096], a_d[:, 2048:4096])

        nc.vector.memset(zeros_w[:], 0.0)
        nc.vector.memset(ones_col[:], 1.0)
        nc.vector.memset(rs_d[:], 0.0)
        nc.vector.memset(rs_a[:], 0.0)

        with tc.tile_pool(name="warm_ps", bufs=1, space="PSUM") as warm_ps:
            warm = warm_ps.tile([C, 256], _F32)
            for _ in range(5):
                nc.tensor.matmul(
                    warm[:], lhsT=zeros_w[:, 0:QT], rhs=zeros_w[:],
                    start=True, stop=True,
                )

        # Head pools take the low PSUM banks; they are freed by ~8us so
        # the main loop's first M tiles can reuse them with no WAR stall.
        head_ctx = contextlib.ExitStack()
        nbc_ps = head_ctx.enter_context(tc.tile_pool(name="nbc_ps", bufs=1, space="PSUM"))
        rrt_ps = head_ctx.enter_context(tc.tile_pool(name="rrt_ps", bufs=1, space="PSUM"))
        bcst_ps = head_ctx.enter_context(tc.tile_pool(name="bcst_ps", bufs=2, space="PSUM"))

        # --- head: norm chain -> D ----------------------------------------
        # nbc col layout: j*8 + u (u<4: b block u, u>=4: c block u-4),
        # blocks are 128 columns of the p-slice chunk j.
        nbc = nbc_ps.tile([C, 16], _F32)

        # |b|,|c| on ACT: abs_max is not a valid TensorScalar ALU op on
        # real TRN2 (codegen rejects it), so activation(Abs) it is.  The
        # [b_j|c_j] packing lets one 1024-col activation cover both.
        def absbc(j):
            base = 1024 * j
            nc.scalar.activation(
                absBC[:, base : base + 1024], bc[:, base : base + 1024],
                _AF.Abs, bias=0.0,
            )

        def norm_chain(j):
            """column L1 norms -> 1/n -> transposed rows (per 512-chunk).
            The DVE links are pinned to a hand schedule so the list
            scheduler cannot interleave them with the D-chain mults."""
            base = 1024 * j
            for u in range(8):
                nc.tensor.matmul(
                    nbc[:, j * 8 + u : j * 8 + u + 1],
                    lhsT=absBC[:, base + u * QT : base + (u + 1) * QT],
                    rhs=ones_col[:],
                    start=True, stop=True,
                )
            with tc.tile_wait_until(0.0050 if j == 0 else 0.0065):
                nc.vector.reciprocal(rr[:, j * 8 : (j + 1) * 8], nbc[:, j * 8 : (j + 1) * 8])
            rrt = rrt_ps.tile([8, C], _BF16, tag="rrt")
            nc.tensor.matmul(
                rrt[:], lhsT=rr[:, j * 8 : (j + 1) * 8], rhs=ident[:],
                start=True, stop=True, is_transpose=True,
            )
            rrT = rrT0 if j == 0 else rrT1
            with tc.tile_wait_until(0.0053 if j == 0 else 0.0068):
                nc.vector.tensor_copy(out=rrT[:], in_=rrt[:])

        def bcast(j):
            """broadcast 1/nb, 1/nc rows across partitions via K=8 matmul
            with one-hot selector weights"""
            rrT = rrT0 if j == 0 else rrT1
            rb_bc = bcst_ps.tile([C, CH], _F32, tag="bcst")
            rc_bc = bcst_ps.tile([C, CH], _F32, tag="bcst")
            for u in range(4):
                nc.tensor.matmul(
                    rb_bc[:, u * QT : (u + 1) * QT],
                    lhsT=onehots[:, u * QT : (u + 1) * QT],
                    rhs=rrT[:],
                    start=True, stop=True,
                )
            for u in range(4):
                nc.tensor.matmul(
                    rc_bc[:, u * QT : (u + 1) * QT],
                    lhsT=onehots[:, (4 + u) * QT : (5 + u) * QT],
                    rhs=rrT[:],
                    start=True, stop=True,
                )
            return rb_bc, rc_bc

        absbc(0)
        norm_chain(0)
        absbc(1)
        norm_chain(1)
        rb0, rc0 = bcast(0)
        rb1, rc1 = bcast(1)
        # D_j = b*rb - c*rc, all on DVE (GPSIMD cannot read the PSUM
        # broadcasts).  Chunk 0 first so the early q-tiles' first matmuls
        # can start against D0 while chunk 1 finishes.
        with tc.tile_wait_until(0.0058):
            nc.vector.tensor_tensor(out=t1a[:], in0=bc[:, 0:CH], in1=rb0[:], op=_OP.mult)
        with tc.tile_wait_until(0.0064):
            nc.vector.tensor_tensor(out=t2a[:], in0=bc[:, CH:1024], in1=rc0[:], op=_OP.mult)
        with tc.tile_wait_until(0.0071):
            nc.vector.tensor_tensor(out=D[:, 0:CH], in0=t1a[:], in1=t2a[:], op=_OP.subtract)
        with tc.tile_wait_until(0.0075):
            nc.vector.tensor_tensor(out=t1b[:], in0=bc[:, 1024 : 1024 + CH], in1=rb1[:], op=_OP.mult)
        with tc.tile_wait_until(0.0082):
            nc.vector.tensor_tensor(out=t2b[:], in0=bc[:, 1024 + CH : 2048], in1=rc1[:], op=_OP.mult)
        with tc.tile_wait_until(0.0089):
            nc.vector.tensor_tensor(out=D[:, CH:PSL], in0=t1b[:], in1=t2b[:], op=_OP.subtract)

        # --- main loop ----------------------------------------------------
        def emit_reduce(eng, m_ap, rs_tile, t):
            w = m_ap.shape[-1]
            if eng == "D":
                nc.vector.tensor_reduce(
                    out=rs_tile[:, t : t + 1], in_=m_ap, axis=_AX.X,
                    op=_OP.add, apply_absolute_value=True,
                )
            elif eng == "A":
                nc.scalar.activation(
                    trash_a[:, 0:w], m_ap, _AF.Abs, bias=0.0,
                    accum_out=rs_tile[:, t : t + 1],
                )

        def rs_of(eng):
            return {"D": rs_d, "A": rs_a}[eng]

        head_ctx.close()

        # |a| on ACT (the only engine with an abs): two 2048-wide
        # activations matching the two a-DMA pieces -- wider slices
        # amortize the per-instruction fixed costs, pulling ACT's first
        # reduce earlier
        for lo, hi in ((0, 2048), (2048, 4096)):
            nc.scalar.activation(
                absA[:, lo:hi], A[:, lo:hi], _AF.Abs, bias=0.0,
            )

        # main loop: 32 full 1024-col tiles through a 4-buf PSUM pipeline.
        # The first EARLY tiles emit their D-chunk-0 matmul up front so the
        # PE has work as soon as D0 lands (~1.6us before D1).
        EARLY = 2
        with tc.tile_pool(name="m_ps", bufs=4, space="PSUM") as m_ps:
            early_tiles = []
            for t in range(EARLY):
                M = m_ps.tile([C, PSL], _F32, tag="m")
                nc.tensor.matmul(
                    M[:, 0:CH], lhsT=A[:, t * QT : (t + 1) * QT],
                    rhs=D[:, 0:CH], start=True, stop=True,
                )
                early_tiles.append(M)
            for t in range(NQT):
                if t < EARLY:
                    M = early_tiles[t]
                else:
                    M = m_ps.tile([C, PSL], _F32, tag="m")
                for j in range(0 if t >= EARLY else 1, NCHK):
                    nc.tensor.matmul(
                        M[:, j * CH : (j + 1) * CH],
                        lhsT=A[:, t * QT : (t + 1) * QT],
                        rhs=D[:, j * CH : (j + 1) * CH],
                        start=True, stop=True,
                    )
                e = fulls[t]
                emit_reduce(e, M[:], rs_of(e), t)

            # --- tail: na matmuls into a recycled pool tile (no pool-close
            # barrier), then combine, scale by 1/na, store ----------------
            na = m_ps.tile([C, PSL], _F32, tag="m")
            for t in range(NQT):
                nc.tensor.matmul(
                    na[:, t : t + 1],
                    lhsT=absA[:, t * QT : (t + 1) * QT],
                    rhs=ones_col[:],
                    start=True, stop=True,
                )
            nc.vector.reciprocal(rna[:], na[:, 0:NQT])
            nc.vector.tensor_tensor(out=sum1[:], in0=rs_d[:], in1=rs_a[:], op=_OP.add)
            nc.vector.tensor_tensor(out=res[:], in0=sum1[:], in1=rna[:], op=_OP.mult)
            nc.sync.dma_start(o_d, res[:])


def _build():
    nc = bacc.Bacc(
        "TRN2", target_bir_lowering=False, debug=False, num_devices=N_CORES
    )
    a_d = nc.dram_tensor("a_full", (C, HW), _BF16, kind="ExternalInput").ap()
    bc_d = nc.dram_tensor("bc", (C, 2 * PSL), _BF16, kind="ExternalInput").ap()
    oh_d = nc.dram_tensor("oh", (8, 8 * QT), _BF16, kind="ExternalInput").ap()
    id_d = nc.dram_tensor("ident", (C, C), _BF16, kind="ExternalInput").ap()
    o_d = nc.dram_tensor("out", (C, NQT), _F32, kind="ExternalOutput").ap()
    with tile.TileContext(nc) as tc:
        _emit(tc, a_d, bc_d, oh_d, id_d, o_d)
    nc.finalize()
    return nc


_NC_CACHE = {}


def _get_nc():
    if "nc" not in _NC_CACHE:
        _NC_CACHE["nc"] = _build()
    return _NC_CACHE["nc"]


def _bf16(x):
    import ml_dtypes

    return np.ascontiguousarray(x.astype(ml_dtypes.bfloat16))


def _in_maps(a, b, c):
    a = np.asarray(a, dtype=np.float32).reshape(B, C, HW)
    b = np.asarray(b, dtype=np.float32).reshape(B, C, HW)
    c = np.asarray(c, dtype=np.float32).reshape(B, C, HW)
    maps = []
    for core in range(N_CORES):
        bi, pi = divmod(core, 4)
        s0 = pi * PSL
        bc = np.concatenate(
            [
                b[bi, :, s0 : s0 + CH],
                c[bi, :, s0 : s0 + CH],
                b[bi, :, s0 + CH : s0 + PSL],
                c[bi, :, s0 + CH : s0 + PSL],
            ],
            axis=1,
        )
        maps.append(
            {
                "a_full": _bf16(a[bi]),
                "bc": _bf16(bc),
                "oh": _bf16(_onehot_const()),
                "ident": _bf16(np.eye(C, dtype=np.float32)),
            }
        )
    return maps


def _onehot_const():
    oh = np.zeros((8, 8 * QT), dtype=np.float32)
    for u in range(8):
        oh[u, u * QT : (u + 1) * QT] = 1.0
    return oh


def kernel(a, b, c):
    nc = _get_nc()
    res = bass_utils.run_bass_kernel_spmd(
        nc, _in_maps(a, b, c), core_ids=list(range(N_CORES))
    )
    total = np.float64(0.0)
    for core in range(N_CORES):
        total += np.sum(res.results[core]["out"], dtype=np.float64)
    return np.float32(total / (B * HW * HW))

